# revision 4
# baseline (speedup 1.0000x reference)
"""Multi-head attention (B=8, N=1024, C=768, H=12) on 8 Trainium2 NeuronCores.

Sharding: data-parallel, one batch element per core. Each core computes the
full attention block for its batch: QKV projection, per-head softmax(QK^T/8)V,
and the output projection, entirely on-chip (SBUF/PSUM) in fp32.

Layout strategy (all chosen so no on-device transposes are needed):
  - host passes x^T [C, N], w_qkv^T [C, 3C], w_proj^T [C, C], bias replicated
    to [128, C].
  - Q, K are produced transposed ([d, n], head-dim on partitions) by the QKV
    matmul; V is produced in natural [n, d] layout by swapping lhsT/rhs.
  - scores are computed transposed (S^T[m, n] = K Q^T) so that exp(S^T) can be
    consumed directly as the moving operand of the P@V matmul.
  - V tiles carry an appended ones-column, so the P@V matmul's 65th output row
    is the softmax denominator (row-sum of exp scores) for free.
  - normalization multiplies by a reciprocal row broadcast across partitions
    via an SBUF->SBUF DMA.
"""

import sys

import numpy as np

if "/opt/trn_rl_repo" not in sys.path:
    sys.path.insert(0, "/opt/trn_rl_repo")

B = 8
N = 1024
C = 768
H = 12
D = 64
SCALE = D ** -0.5
KT = C // 128           # 6 contraction tiles over channels
MT_QK = 2 * C // 128    # 12 output tiles for Q and K (o in [0, 1536))
NT = N // 128           # 8 token tiles
PAIRS = H // 2          # 6 head pairs

_CACHE = {}


def build_program():
    import concourse.bacc as bacc
    import concourse.mybir as mybir
    import concourse.tile as tile

    f32 = mybir.dt.float32
    Exp = mybir.ActivationFunctionType.Exp

    nc = bacc.Bacc("TRN2", target_bir_lowering=False, debug=False)

    xT_d = nc.dram_tensor("xT", [C, N], f32, kind="ExternalInput")
    wqkvT_d = nc.dram_tensor("wqkvT", [C, 3 * C], f32, kind="ExternalInput")
    wprojT_d = nc.dram_tensor("wprojT", [C, C], f32, kind="ExternalInput")
    bias_d = nc.dram_tensor("bias_rep", [128, C], f32, kind="ExternalInput")
    y_d = nc.dram_tensor("y", [N, C], f32, kind="ExternalOutput")

    with tile.TileContext(nc) as tc:
        with tc.tile_pool(name="pers", bufs=1) as pers:
            # Q^T,K^T tiles [d, n]: tile m holds heads 2m (parts 0:64) and
            # 2m+1 (parts 64:128); m 0..5 = Q, 6..11 = K.
            qkt = [pers.tile([128, N], f32, name=f"qkt{m}", tag=f"qkt{m}")
                   for m in range(MT_QK)]
            # V tiles [n-tile, pair, 130]: per pair block [V_h0 |1| V_h1 |1];
            # ones cols at 64 and 129 feed the denominator row of P@V.
            vbuf = [pers.tile([128, PAIRS, 130], f32, name=f"vbuf{i}", tag=f"vbuf{i}")
                    for i in range(NT)]

            # ---------------- phase A: QKV projection ----------------
            with tc.tile_pool(name="phA", bufs=1) as pA, \
                 tc.tile_pool(name="phA_ps", bufs=2, space="PSUM") as pAp:
                xt = [pA.tile([128, N], f32, name=f"xt{k}", tag=f"xt{k}")
                      for k in range(KT)]
                wq = [pA.tile([128, 3 * C], f32, name=f"wq{k}", tag=f"wq{k}")
                      for k in range(KT)]
                for k in range(KT):
                    nc.sync.dma_start(xt[k][:], xT_d[128 * k:128 * (k + 1), :])
                    nc.sync.dma_start(wq[k][:], wqkvT_d[128 * k:128 * (k + 1), :])
                for i in range(NT):
                    ones_ap = vbuf[i].rearrange("p a (t c) -> p a t c", c=65)[:, :, :, 64]
                    nc.vector.memset(ones_ap, 1.0)

                # Q^T / K^T: out[o-tile, n] = wqkvT[:, o-cols].T @ x^T
                for m in range(MT_QK):
                    ps = pAp.tile([128, N], f32, name="qk_ps", tag="qk_ps")
                    for j in range(2):
                        for k in range(KT):
                            nc.tensor.matmul(
                                ps[:, 512 * j:512 * (j + 1)],
                                wq[k][:, 128 * m:128 * (m + 1)],
                                xt[k][:, 512 * j:512 * (j + 1)],
                                start=(k == 0), stop=(k == KT - 1),
                            )
                    nc.scalar.copy(qkt[m][:], ps[:])

                # V natural: out[n-tile, o_v] = x^T[:, n-cols].T @ wqkvT[:, v-cols]
                for i in range(NT):
                    ps = pAp.tile([128, N], f32, name="v_ps", tag="v_ps")
                    for c0, w in ((0, 512), (512, 256)):
                        for k in range(KT):
                            nc.tensor.matmul(
                                ps[:, c0:c0 + w],
                                xt[k][:, 128 * i:128 * (i + 1)],
                                wq[k][:, 2 * C + c0:2 * C + c0 + w],
                                start=(k == 0), stop=(k == KT - 1),
                            )
                    v_view = ps[:, 0:C].rearrange("p (a t c) -> p a t c", t=2, c=64)
                    nc.vector.tensor_copy(vbuf[i][:, :, 0:64], v_view[:, :, 0, :])
                    nc.vector.tensor_copy(vbuf[i][:, :, 65:129], v_view[:, :, 1, :])

            # ---------------- phase B: attention + projection ----------------
            with tc.tile_pool(name="phB1", bufs=1) as pB1, \
                 tc.tile_pool(name="phB", bufs=2) as pB, \
                 tc.tile_pool(name="dramb", bufs=2, space="DRAM") as pDr, \
                 tc.tile_pool(name="s_ps_pool", bufs=1, space="PSUM") as pBs, \
                 tc.tile_pool(name="pv_ps_pool", bufs=2, space="PSUM") as pBpv, \
                 tc.tile_pool(name="proj_ps_pool", bufs=2, space="PSUM") as pPj:
                # attn_out^T tiles [c, n]; wproj/bias loaded into reused space.
                aot = [pB1.tile([128, N], f32, name=f"aot{t}", tag=f"aot{t}")
                       for t in range(PAIRS)]
                wp = [pB1.tile([128, C], f32, name=f"wp{k}", tag=f"wp{k}")
                      for k in range(KT)]
                bias_t = pB1.tile([128, C], f32, name="bias_t", tag="bias_t")
                for k in range(KT):
                    nc.sync.dma_start(wp[k][:], wprojT_d[128 * k:128 * (k + 1), :])
                nc.sync.dma_start(bias_t[:], bias_d[:])

                for t in range(PAIRS):
                    qt, kt = qkt[t], qkt[PAIRS + t]
                    pv_ps = [pBpv.tile([65, N], f32, name=f"pv{h}", tag="pv")
                             for h in range(2)]
                    for i in range(NT):
                        stexp = pB.tile([128, 2, N], f32, name="stexp", tag="stexp")
                        for j in range(2):
                            s_ps = pBs.tile([128, N], f32, name="s_ps", tag="s_ps")
                            for h in range(2):
                                # S^T[m, n] = sum_d K^T[d, m] * Q^T[d, n]
                                nc.tensor.matmul(
                                    s_ps[:, 512 * h:512 * (h + 1)],
                                    kt[64 * h:64 * (h + 1), 128 * i:128 * (i + 1)],
                                    qt[64 * h:64 * (h + 1), 512 * j:512 * (j + 1)],
                                    start=True, stop=True,
                                )
                            # exp(S^T / 8) for both heads in one pass, PSUM -> SBUF
                            nc.scalar.activation(
                                stexp[:, :, 512 * j:512 * (j + 1)],
                                s_ps[:, 0:N].rearrange("p (h n) -> p h n", h=2),
                                Exp, scale=SCALE,
                            )
                        for h in range(2):
                            for j in range(2):
                                # (exp(S^T) stacked with ones-col V): rows 0:64 are
                                # (P~ @ V)^T, row 64 is the softmax denominator.
                                nc.tensor.matmul(
                                    pv_ps[h][:, 512 * j:512 * (j + 1)],
                                    vbuf[i][:, t, 65 * h:65 * (h + 1)],
                                    stexp[:, h, 512 * j:512 * (j + 1)],
                                    start=(i == 0), stop=(i == NT - 1),
                                )
                    for h in range(2):
                        stage = pB.tile([65, N], f32, name="stage", tag="stage")
                        nc.vector.tensor_copy(stage[:], pv_ps[h][:])
                        nc.vector.reciprocal(stage[64:65, :], stage[64:65, :])
                        # partition-broadcast of the reciprocal row: SBUF APs
                        # can't have zero partition step, so bounce via DRAM
                        # (DRAM sources may broadcast).
                        dr = pDr.tile([1, N], f32, name="dr", tag="dr")
                        nc.sync.dma_start(dr[:], stage[64:65, :])
                        rb = pB.tile([64, N], f32, name="rb", tag="rb")
                        nc.sync.dma_start(rb[:], dr[:].to_broadcast((64, N)))
                        if h == 0:
                            nc.vector.tensor_mul(aot[t][0:64, :], stage[0:64, :], rb[:])
                        else:
                            tmp = pB.tile([64, N], f32, name="tmp1", tag="tmp1")
                            nc.vector.tensor_mul(tmp[:], stage[0:64, :], rb[:])
                            # DVE lanes cannot shift partitions; DMA moves the
                            # odd head's rows into partitions 64:128.
                            nc.sync.dma_start(aot[t][64:128, :], tmp[:])

                # output projection: y[n, o] = attn_out^T.T @ w_proj^T + b
                for i in range(NT):
                    yt = pB.tile([128, C], f32, name="yt", tag="yt")
                    for c0 in (0, 384):
                        pp = pPj.tile([128, 384], f32, name="pp", tag="pp")
                        for k in range(KT):
                            nc.tensor.matmul(
                                pp[:, 0:384],
                                aot[k][:, 128 * i:128 * (i + 1)],
                                wp[k][:, c0:c0 + 384],
                                start=(k == 0), stop=(k == KT - 1),
                            )
                        nc.vector.tensor_add(yt[:, c0:c0 + 384], pp[:, 0:384],
                                             bias_t[:, c0:c0 + 384])
                    nc.sync.dma_start(y_d[128 * i:128 * (i + 1), :], yt[:])

    nc.compile()
    return nc


def make_in_maps(x, w_qkv, w_proj, b_proj):
    wqkvT = np.ascontiguousarray(np.asarray(w_qkv, dtype=np.float32).T)
    wprojT = np.ascontiguousarray(np.asarray(w_proj, dtype=np.float32).T)
    bias_rep = np.ascontiguousarray(
        np.broadcast_to(np.asarray(b_proj, dtype=np.float32), (128, C)))
    x = np.asarray(x, dtype=np.float32)
    return [
        {
            "xT": np.ascontiguousarray(x[b].T),
            "wqkvT": wqkvT,
            "wprojT": wprojT,
            "bias_rep": bias_rep,
        }
        for b in range(B)
    ]


def kernel(x, w_qkv, w_proj, b_proj):
    from concourse.bass_utils import run_bass_kernel_spmd

    if "nc" not in _CACHE:
        _CACHE["nc"] = build_program()
    nc = _CACHE["nc"]

    in_maps = make_in_maps(x, w_qkv, w_proj, b_proj)
    res = run_bass_kernel_spmd(nc, in_maps, core_ids=list(range(B)))
    out = np.stack([res.results[b]["y"] for b in range(B)], axis=0)
    return out.astype(np.float32)


# revision 9
# speedup vs baseline: 1.8109x; 1.8109x over previous
"""Multi-head attention (B=8, N=1024, C=768, H=12) on 8 Trainium2 NeuronCores.

Sharding: data-parallel, one batch element per core. Each core computes the
full attention block for its batch: QKV projection, per-head softmax(QK^T/8)V,
and the output projection, entirely on-chip (SBUF/PSUM) in fp32.

Layout strategy (all chosen so no on-device transposes are needed):
  - host passes x^T [C, N], w_qkv^T [C, 3C], w_proj^T [C, C], bias replicated
    to [128, C].
  - Q, K are produced transposed ([d, n], head-dim on partitions) by the QKV
    matmul; V is produced in natural [n, d] layout by swapping lhsT/rhs.
  - scores are computed transposed (S^T[m, n] = K Q^T) so that exp(S^T) can be
    consumed directly as the moving operand of the P@V matmul.
  - V tiles carry an appended ones-column, so the P@V matmul's 65th output row
    is the softmax denominator (row-sum of exp scores) for free.
  - normalization multiplies by a reciprocal row broadcast across partitions
    via an SBUF->SBUF DMA.
"""

import sys

import numpy as np

if "/opt/trn_rl_repo" not in sys.path:
    sys.path.insert(0, "/opt/trn_rl_repo")

B = 8
N = 1024
C = 768
H = 12
D = 64
SCALE = D ** -0.5
KT = C // 128           # 6 contraction tiles over channels
MT_QK = 2 * C // 128    # 12 output tiles for Q and K (o in [0, 1536))
NT = N // 128           # 8 token tiles
PAIRS = H // 2          # 6 head pairs

_CACHE = {}


def build_program(fast=True):
    import concourse.bacc as bacc
    import concourse.mybir as mybir
    import concourse.tile as tile

    f32 = mybir.dt.float32
    f32r = mybir.dt.float32r
    Exp = mybir.ActivationFunctionType.Exp

    # fm: dtype for all matmul operands. float32r streams one output column
    # per cycle (vs 4 for float32's two-pass LOW_HIGH emulation) and keeps
    # 11 mantissa bits; inputs must be produced pre-rounded to FP32r.
    fm = f32r if fast else f32

    def mm(out, lhsT, rhs, start, stop, fast=True):
        nc.tensor.matmul(out, lhsT, rhs, start=start, stop=stop)

    nc = bacc.Bacc("TRN2", target_bir_lowering=False, debug=False)

    xT_d = nc.dram_tensor("xT", [C, N], fm, kind="ExternalInput")
    wqkvT_d = nc.dram_tensor("wqkvT", [C, 3 * C], fm, kind="ExternalInput")
    wprojT_d = nc.dram_tensor("wprojT", [C, C], fm, kind="ExternalInput")
    bias_d = nc.dram_tensor("bias_rep", [128, C], f32, kind="ExternalInput")
    y_d = nc.dram_tensor("y", [N, C], f32, kind="ExternalOutput")

    with tile.TileContext(nc) as tc:
        with tc.tile_pool(name="pers", bufs=1) as pers:
            # Q^T,K^T tiles [d, n]: tile m holds heads 2m (parts 0:64) and
            # 2m+1 (parts 64:128); m 0..5 = Q, 6..11 = K.
            qkt = [pers.tile([128, N], fm, name=f"qkt{m}", tag=f"qkt{m}")
                   for m in range(MT_QK)]
            # V tiles [n-tile, pair, 130]: per pair block [V_h0 |1| V_h1 |1];
            # ones cols at 64 and 129 feed the denominator row of P@V.
            vbuf = [pers.tile([128, PAIRS, 130], fm, name=f"vbuf{i}", tag=f"vbuf{i}")
                    for i in range(NT)]

            # ---------------- phase A: QKV projection ----------------
            with tc.tile_pool(name="phA", bufs=1) as pA, \
                 tc.tile_pool(name="phA_ps", bufs=2, space="PSUM") as pAp:
                xt = [pA.tile([128, N], fm, name=f"xt{k}", tag=f"xt{k}")
                      for k in range(KT)]
                wq = [pA.tile([128, 3 * C], fm, name=f"wq{k}", tag=f"wq{k}")
                      for k in range(KT)]
                for k in range(KT):
                    nc.sync.dma_start(xt[k][:], xT_d[128 * k:128 * (k + 1), :])
                    nc.sync.dma_start(wq[k][:], wqkvT_d[128 * k:128 * (k + 1), :])
                for i in range(NT):
                    ones_ap = vbuf[i].rearrange("p a (t c) -> p a t c", c=65)[:, :, :, 64]
                    nc.vector.memset(ones_ap.bitcast(f32), 1.0)

                # Q^T / K^T: out[o-tile, n] = wqkvT[:, o-cols].T @ x^T
                for m in range(MT_QK):
                    ps = pAp.tile([128, N], f32, name="qk_ps", tag="qk_ps")
                    for j in range(2):
                        for k in range(KT):
                            mm(
                                ps[:, 512 * j:512 * (j + 1)],
                                wq[k][:, 128 * m:128 * (m + 1)],
                                xt[k][:, 512 * j:512 * (j + 1)],
                                start=(k == 0), stop=(k == KT - 1),
                                                            )
                    nc.scalar.copy(qkt[m][:], ps[:])

                # V natural: out[n-tile, o_v] = x^T[:, n-cols].T @ wqkvT[:, v-cols]
                for i in range(NT):
                    ps = pAp.tile([128, N], f32, name="v_ps", tag="v_ps")
                    for c0, w in ((0, 512), (512, 256)):
                        for k in range(KT):
                            mm(
                                ps[:, c0:c0 + w],
                                xt[k][:, 128 * i:128 * (i + 1)],
                                wq[k][:, 2 * C + c0:2 * C + c0 + w],
                                start=(k == 0), stop=(k == KT - 1),
                                                            )
                    v_view = ps[:, 0:C].rearrange("p (a t c) -> p a t c", t=2, c=64)
                    nc.vector.tensor_copy(vbuf[i][:, :, 0:64], v_view[:, :, 0, :])
                    nc.vector.tensor_copy(vbuf[i][:, :, 65:129], v_view[:, :, 1, :])

            # ---------------- phase B: attention + projection ----------------
            with tc.tile_pool(name="phB1", bufs=1) as pB1, \
                 tc.tile_pool(name="phB", bufs=2) as pB, \
                 tc.tile_pool(name="dramb", bufs=2, space="DRAM") as pDr, \
                 tc.tile_pool(name="s_ps_pool", bufs=1, space="PSUM") as pBs, \
                 tc.tile_pool(name="pv_ps_pool", bufs=2, space="PSUM") as pBpv, \
                 tc.tile_pool(name="proj_ps_pool", bufs=2, space="PSUM") as pPj:
                # attn_out^T tiles [c, n]; wproj/bias loaded into reused space.
                aot = [pB1.tile([128, N], fm, name=f"aot{t}", tag=f"aot{t}")
                       for t in range(PAIRS)]
                wp = [pB1.tile([128, C], fm, name=f"wp{k}", tag=f"wp{k}")
                      for k in range(KT)]
                bias_t = pB1.tile([128, C], f32, name="bias_t", tag="bias_t")
                for k in range(KT):
                    nc.sync.dma_start(wp[k][:], wprojT_d[128 * k:128 * (k + 1), :])
                nc.sync.dma_start(bias_t[:], bias_d[:])

                for t in range(PAIRS):
                    qt, kt = qkt[t], qkt[PAIRS + t]
                    pv_ps = [pBpv.tile([65, N], f32, name=f"pv{h}", tag="pv")
                             for h in range(2)]
                    for i in range(NT):
                        stexp = pB.tile([128, 2, N], fm, name="stexp", tag="stexp")
                        for j in range(2):
                            s_ps = pBs.tile([128, N], f32, name="s_ps", tag="s_ps")
                            for h in range(2):
                                # S^T[m, n] = sum_d K^T[d, m] * Q^T[d, n]
                                mm(
                                    s_ps[:, 512 * h:512 * (h + 1)],
                                    kt[64 * h:64 * (h + 1), 128 * i:128 * (i + 1)],
                                    qt[64 * h:64 * (h + 1), 512 * j:512 * (j + 1)],
                                    start=True, stop=True,
                                )
                            # exp(S^T / 8) for both heads in one pass, PSUM -> SBUF
                            nc.scalar.activation(
                                stexp[:, :, 512 * j:512 * (j + 1)],
                                s_ps[:, 0:N].rearrange("p (h n) -> p h n", h=2),
                                Exp, scale=SCALE,
                            )
                        for h in range(2):
                            for j in range(2):
                                # (exp(S^T) stacked with ones-col V): rows 0:64 are
                                # (P~ @ V)^T, row 64 is the softmax denominator.
                                mm(
                                    pv_ps[h][:, 512 * j:512 * (j + 1)],
                                    vbuf[i][:, t, 65 * h:65 * (h + 1)],
                                    stexp[:, h, 512 * j:512 * (j + 1)],
                                    start=(i == 0), stop=(i == NT - 1),
                                                                    )
                    for h in range(2):
                        stage = pB.tile([65, N], f32, name="stage", tag="stage")
                        nc.vector.tensor_copy(stage[:], pv_ps[h][:])
                        nc.vector.reciprocal(stage[64:65, :], stage[64:65, :])
                        # partition-broadcast of the reciprocal row: SBUF APs
                        # can't have zero partition step, so bounce via DRAM
                        # (DRAM sources may broadcast).
                        dr = pDr.tile([1, N], f32, name="dr", tag="dr")
                        nc.sync.dma_start(dr[:], stage[64:65, :])
                        rb = pB.tile([64, N], f32, name="rb", tag="rb")
                        nc.sync.dma_start(rb[:], dr[:].to_broadcast((64, N)))
                        if h == 0:
                            nc.vector.tensor_mul(aot[t][0:64, :], stage[0:64, :], rb[:])
                        else:
                            tmp = pB.tile([64, N], fm, name="tmp1", tag="tmp1")
                            nc.vector.tensor_mul(tmp[:], stage[0:64, :], rb[:])
                            # DVE lanes cannot shift partitions; DMA moves the
                            # odd head's rows into partitions 64:128.
                            nc.sync.dma_start(aot[t][64:128, :], tmp[:])

                # output projection: y[n, o] = attn_out^T.T @ w_proj^T + b
                for i in range(NT):
                    yt = pB.tile([128, C], f32, name="yt", tag="yt")
                    for c0 in (0, 384):
                        pp = pPj.tile([128, 384], f32, name="pp", tag="pp")
                        for k in range(KT):
                            mm(
                                pp[:, 0:384],
                                aot[k][:, 128 * i:128 * (i + 1)],
                                wp[k][:, c0:c0 + 384],
                                start=(k == 0), stop=(k == KT - 1),
                                                            )
                        nc.vector.tensor_add(yt[:, c0:c0 + 384], pp[:, 0:384],
                                             bias_t[:, c0:c0 + 384])
                    nc.sync.dma_start(y_d[128 * i:128 * (i + 1), :], yt[:])

    nc.compile()
    return nc


def round_f32r(a):
    """Round fp32 to the FP32r grid (11 explicit mantissa bits, RNE) --
    what the PE reads for float32r matmuls."""
    a = np.ascontiguousarray(a, dtype=np.float32)
    b = a.view(np.uint32)
    r = (b + np.uint32(0x7FF) + ((b >> np.uint32(12)) & np.uint32(1))) \
        & np.uint32(0xFFFFF000)
    return r.view(np.float32)


def make_in_maps(x, w_qkv, w_proj, b_proj):
    wqkvT = round_f32r(np.asarray(w_qkv, dtype=np.float32).T)
    wprojT = round_f32r(np.asarray(w_proj, dtype=np.float32).T)
    bias_rep = np.ascontiguousarray(
        np.broadcast_to(np.asarray(b_proj, dtype=np.float32), (128, C)))
    x = np.asarray(x, dtype=np.float32)
    return [
        {
            "xT": round_f32r(x[b].T),
            "wqkvT": wqkvT,
            "wprojT": wprojT,
            "bias_rep": bias_rep,
        }
        for b in range(B)
    ]


def kernel(x, w_qkv, w_proj, b_proj):
    from concourse.bass_utils import run_bass_kernel_spmd

    if "nc" not in _CACHE:
        _CACHE["nc"] = build_program()
    nc = _CACHE["nc"]

    in_maps = make_in_maps(x, w_qkv, w_proj, b_proj)
    res = run_bass_kernel_spmd(nc, in_maps, core_ids=list(range(B)))
    out = np.stack([res.results[b]["y"] for b in range(B)], axis=0)
    return out.astype(np.float32)


# revision 11
# speedup vs baseline: 2.7045x; 1.4934x over previous
"""Multi-head attention (B=8, N=1024, C=768, H=12) on 8 Trainium2 NeuronCores.

Sharding: data-parallel, one batch element per core. Each core computes the
full attention block for its batch: QKV projection, per-head softmax(QK^T/8)V,
and the output projection, entirely on-chip (SBUF/PSUM) in fp32.

Layout strategy (all chosen so no on-device transposes are needed):
  - host passes x^T [C, N], w_qkv^T [C, 3C], w_proj^T [C, C], bias replicated
    to [128, C].
  - Q, K are produced transposed ([d, n], head-dim on partitions) by the QKV
    matmul; V is produced in natural [n, d] layout by swapping lhsT/rhs.
  - scores are computed transposed (S^T[m, n] = K Q^T) so that exp(S^T) can be
    consumed directly as the moving operand of the P@V matmul.
  - V tiles carry an appended ones-column, so the P@V matmul's 65th output row
    is the softmax denominator (row-sum of exp scores) for free.
  - normalization multiplies by a reciprocal row broadcast across partitions
    via an SBUF->SBUF DMA.
"""

import sys

import numpy as np

if "/opt/trn_rl_repo" not in sys.path:
    sys.path.insert(0, "/opt/trn_rl_repo")

B = 8
N = 1024
C = 768
H = 12
D = 64
SCALE = D ** -0.5
KT = C // 128           # 6 contraction tiles over channels
MT_QK = 2 * C // 128    # 12 output tiles for Q and K (o in [0, 1536))
NT = N // 128           # 8 token tiles
PAIRS = H // 2          # 6 head pairs

_CACHE = {}


def build_program(fast=True):
    import concourse.bacc as bacc
    import concourse.mybir as mybir
    import concourse.tile as tile

    f32 = mybir.dt.float32
    f32r = mybir.dt.float32r
    Exp = mybir.ActivationFunctionType.Exp

    # fm: dtype for all matmul operands. float32r streams one output column
    # per cycle (vs 4 for float32's two-pass LOW_HIGH emulation) and keeps
    # 11 mantissa bits; inputs must be produced pre-rounded to FP32r.
    fm = f32r if fast else f32

    def mm(out, lhsT, rhs, start, stop, fast=True):
        nc.tensor.matmul(out, lhsT, rhs, start=start, stop=stop)

    nc = bacc.Bacc("TRN2", target_bir_lowering=False, debug=False)

    xT_d = nc.dram_tensor("xT", [C, N], fm, kind="ExternalInput")
    wqkvT_d = nc.dram_tensor("wqkvT", [C, 3 * C], fm, kind="ExternalInput")
    wprojT_d = nc.dram_tensor("wprojT", [C, C], fm, kind="ExternalInput")
    bias_d = nc.dram_tensor("bias_rep", [128, C], f32, kind="ExternalInput")
    y_d = nc.dram_tensor("y", [N, C], f32, kind="ExternalOutput")

    with tile.TileContext(nc) as tc:
        with tc.tile_pool(name="pers", bufs=1) as pers:
            # Q^T,K^T tiles [d, n]: tile m holds heads 2m (parts 0:64) and
            # 2m+1 (parts 64:128); m 0..5 = Q, 6..11 = K.
            qkt = [pers.tile([128, N], fm, name=f"qkt{m}", tag=f"qkt{m}")
                   for m in range(MT_QK)]
            # V tiles [n-tile, pair, 130]: per pair block [V_h0 |1| V_h1 |1];
            # ones cols at 64 and 129 feed the denominator row of P@V.
            vbuf = [pers.tile([128, PAIRS, 130], fm, name=f"vbuf{i}", tag=f"vbuf{i}")
                    for i in range(NT)]

            # ---------------- phase A: QKV projection ----------------
            with tc.tile_pool(name="phA", bufs=1) as pA, \
                 tc.tile_pool(name="phA_ps", bufs=2, space="PSUM") as pAp:
                xt = [pA.tile([128, N], fm, name=f"xt{k}", tag=f"xt{k}")
                      for k in range(KT)]
                wq = [pA.tile([128, 3 * C], fm, name=f"wq{k}", tag=f"wq{k}")
                      for k in range(KT)]
                for k in range(KT):
                    nc.sync.dma_start(xt[k][:], xT_d[128 * k:128 * (k + 1), :])
                    nc.sync.dma_start(wq[k][:], wqkvT_d[128 * k:128 * (k + 1), :])
                for i in range(NT):
                    ones_ap = vbuf[i].rearrange("p a (t c) -> p a t c", c=65)[:, :, :, 64]
                    nc.vector.memset(ones_ap.bitcast(f32), 1.0)

                # Q^T / K^T: out[o-tile, n] = wqkvT[:, o-cols].T @ x^T
                for m in range(MT_QK):
                    ps = pAp.tile([128, N], f32, name="qk_ps", tag="qk_ps")
                    for j in range(2):
                        for k in range(KT):
                            mm(
                                ps[:, 512 * j:512 * (j + 1)],
                                wq[k][:, 128 * m:128 * (m + 1)],
                                xt[k][:, 512 * j:512 * (j + 1)],
                                start=(k == 0), stop=(k == KT - 1),
                                                            )
                    nc.scalar.copy(qkt[m][:], ps[:])

                # V natural: out[n-tile, o_v] = x^T[:, n-cols].T @ wqkvT[:, v-cols]
                for i in range(NT):
                    ps = pAp.tile([128, N], f32, name="v_ps", tag="v_ps")
                    for c0, w in ((0, 512), (512, 256)):
                        for k in range(KT):
                            mm(
                                ps[:, c0:c0 + w],
                                xt[k][:, 128 * i:128 * (i + 1)],
                                wq[k][:, 2 * C + c0:2 * C + c0 + w],
                                start=(k == 0), stop=(k == KT - 1),
                                                            )
                    v_view = ps[:, 0:C].rearrange("p (a t c) -> p a t c", t=2, c=64)
                    nc.vector.tensor_copy(vbuf[i][:, :, 0:64], v_view[:, :, 0, :])
                    nc.vector.tensor_copy(vbuf[i][:, :, 65:129], v_view[:, :, 1, :])

            # ---------------- phase B: attention + projection ----------------
            with tc.tile_pool(name="phB1", bufs=1) as pB1, \
                 tc.tile_pool(name="phB", bufs=2) as pB, \
                 tc.tile_pool(name="dramb", bufs=2, space="DRAM") as pDr, \
                 tc.tile_pool(name="s_ps_pool", bufs=2, space="PSUM") as pBs, \
                 tc.tile_pool(name="pv_ps_pool", bufs=2, space="PSUM") as pBpv:
                # attn_out^T tiles [c, n]; wproj/bias loaded into reused space.
                aot = [pB1.tile([128, N], fm, name=f"aot{t}", tag=f"aot{t}")
                       for t in range(PAIRS)]
                wp = [pB1.tile([128, C], fm, name=f"wp{k}", tag=f"wp{k}")
                      for k in range(KT)]
                bias_t = pB1.tile([128, C], f32, name="bias_t", tag="bias_t")
                for k in range(KT):
                    nc.sync.dma_start(wp[k][:], wprojT_d[128 * k:128 * (k + 1), :])
                nc.sync.dma_start(bias_t[:], bias_d[:])

                for t in range(PAIRS):
                    qt, kt = qkt[t], qkt[PAIRS + t]
                    pv_ps = [pBpv.tile([65, N], f32, name=f"pv{h}", tag="pv")
                             for h in range(2)]
                    for i in range(NT):
                        stexp = pB.tile([128, 2, N], fm, name="stexp", tag="stexp", bufs=3)
                        for j in range(2):
                            s_ps = pBs.tile([128, N], f32, name="s_ps", tag="s_ps")
                            for h in range(2):
                                # S^T[m, n] = sum_d K^T[d, m] * Q^T[d, n]
                                mm(
                                    s_ps[:, 512 * h:512 * (h + 1)],
                                    kt[64 * h:64 * (h + 1), 128 * i:128 * (i + 1)],
                                    qt[64 * h:64 * (h + 1), 512 * j:512 * (j + 1)],
                                    start=True, stop=True,
                                )
                            # exp(S^T / 8) for both heads in one pass, PSUM -> SBUF
                            nc.scalar.activation(
                                stexp[:, :, 512 * j:512 * (j + 1)],
                                s_ps[:, 0:N].rearrange("p (h n) -> p h n", h=2),
                                Exp, scale=SCALE,
                            )
                        for h in range(2):
                            for j in range(2):
                                # (exp(S^T) stacked with ones-col V): rows 0:64 are
                                # (P~ @ V)^T, row 64 is the softmax denominator.
                                mm(
                                    pv_ps[h][:, 512 * j:512 * (j + 1)],
                                    vbuf[i][:, t, 65 * h:65 * (h + 1)],
                                    stexp[:, h, 512 * j:512 * (j + 1)],
                                    start=(i == 0), stop=(i == NT - 1),
                                                                    )
                    for h in range(2):
                        stage = pB.tile([65, N], f32, name="stage", tag="stage")
                        nc.vector.tensor_copy(stage[:], pv_ps[h][:])
                        nc.vector.reciprocal(stage[64:65, :], stage[64:65, :])
                        # partition-broadcast of the reciprocal row: SBUF APs
                        # can't have zero partition step, so bounce via DRAM
                        # (DRAM sources may broadcast).
                        dr = pDr.tile([1, N], f32, name="dr", tag="dr")
                        nc.sync.dma_start(dr[:], stage[64:65, :])
                        rb = pB.tile([64, N], f32, name="rb", tag="rb")
                        nc.sync.dma_start(rb[:], dr[:].to_broadcast((64, N)))
                        if h == 0:
                            nc.vector.tensor_mul(aot[t][0:64, :], stage[0:64, :], rb[:])
                        else:
                            tmp = pB.tile([64, N], fm, name="tmp1", tag="tmp1")
                            nc.vector.tensor_mul(tmp[:], stage[0:64, :], rb[:])
                            # DVE lanes cannot shift partitions; DMA moves the
                            # odd head's rows into partitions 64:128.
                            nc.sync.dma_start(aot[t][64:128, :], tmp[:])

                # output projection: y[n, o] = attn_out^T.T @ w_proj^T + b
                for i in range(NT):
                    yt = pB.tile([128, C], f32, name="yt", tag="yt")
                    for c0 in (0, 384):
                        pp = pBs.tile([128, 384], f32, name="pp", tag="s_ps")
                        for k in range(KT):
                            mm(
                                pp[:, 0:384],
                                aot[k][:, 128 * i:128 * (i + 1)],
                                wp[k][:, c0:c0 + 384],
                                start=(k == 0), stop=(k == KT - 1),
                                                            )
                        nc.vector.tensor_add(yt[:, c0:c0 + 384], pp[:, 0:384],
                                             bias_t[:, c0:c0 + 384])
                    nc.sync.dma_start(y_d[128 * i:128 * (i + 1), :], yt[:])

    nc.compile()
    return nc


def round_f32r(a):
    """Round fp32 to the FP32r grid (11 explicit mantissa bits, RNE) --
    what the PE reads for float32r matmuls."""
    a = np.ascontiguousarray(a, dtype=np.float32)
    b = a.view(np.uint32)
    r = (b + np.uint32(0x7FF) + ((b >> np.uint32(12)) & np.uint32(1))) \
        & np.uint32(0xFFFFF000)
    return r.view(np.float32)


def make_in_maps(x, w_qkv, w_proj, b_proj):
    wqkvT = round_f32r(np.asarray(w_qkv, dtype=np.float32).T)
    wprojT = round_f32r(np.asarray(w_proj, dtype=np.float32).T)
    bias_rep = np.ascontiguousarray(
        np.broadcast_to(np.asarray(b_proj, dtype=np.float32), (128, C)))
    x = np.asarray(x, dtype=np.float32)
    return [
        {
            "xT": round_f32r(x[b].T),
            "wqkvT": wqkvT,
            "wprojT": wprojT,
            "bias_rep": bias_rep,
        }
        for b in range(B)
    ]


def kernel(x, w_qkv, w_proj, b_proj):
    from concourse.bass_utils import run_bass_kernel_spmd

    if "nc" not in _CACHE:
        _CACHE["nc"] = build_program()
    nc = _CACHE["nc"]

    in_maps = make_in_maps(x, w_qkv, w_proj, b_proj)
    res = run_bass_kernel_spmd(nc, in_maps, core_ids=list(range(B)))
    out = np.stack([res.results[b]["y"] for b in range(B)], axis=0)
    return out.astype(np.float32)


# revision 12
# speedup vs baseline: 2.9379x; 1.0863x over previous
"""Multi-head attention (B=8, N=1024, C=768, H=12) on 8 Trainium2 NeuronCores.

Sharding: data-parallel, one batch element per core. Each core computes the
full attention block for its batch: QKV projection, per-head softmax(QK^T/8)V,
and the output projection, entirely on-chip (SBUF/PSUM) in fp32.

Layout strategy (all chosen so no on-device transposes are needed):
  - host passes x^T [C, N], w_qkv^T [C, 3C], w_proj^T [C, C], bias replicated
    to [128, C].
  - Q, K are produced transposed ([d, n], head-dim on partitions) by the QKV
    matmul; V is produced in natural [n, d] layout by swapping lhsT/rhs.
  - scores are computed transposed (S^T[m, n] = K Q^T) so that exp(S^T) can be
    consumed directly as the moving operand of the P@V matmul.
  - V tiles carry an appended ones-column, so the P@V matmul's 65th output row
    is the softmax denominator (row-sum of exp scores) for free.
  - normalization multiplies by a reciprocal row broadcast across partitions
    via an SBUF->SBUF DMA.
"""

import sys

import numpy as np

if "/opt/trn_rl_repo" not in sys.path:
    sys.path.insert(0, "/opt/trn_rl_repo")

B = 8
N = 1024
C = 768
H = 12
D = 64
SCALE = D ** -0.5
KT = C // 128           # 6 contraction tiles over channels
MT_QK = 2 * C // 128    # 12 output tiles for Q and K (o in [0, 1536))
NT = N // 128           # 8 token tiles
PAIRS = H // 2          # 6 head pairs

_CACHE = {}


def build_program(fast=True):
    import concourse.bacc as bacc
    import concourse.mybir as mybir
    import concourse.tile as tile

    f32 = mybir.dt.float32
    f32r = mybir.dt.float32r
    Exp = mybir.ActivationFunctionType.Exp

    # fm: dtype for all matmul operands. float32r streams one output column
    # per cycle (vs 4 for float32's two-pass LOW_HIGH emulation) and keeps
    # 11 mantissa bits; inputs must be produced pre-rounded to FP32r.
    fm = f32r if fast else f32

    def mm(out, lhsT, rhs, start, stop, fast=True):
        nc.tensor.matmul(out, lhsT, rhs, start=start, stop=stop)

    nc = bacc.Bacc("TRN2", target_bir_lowering=False, debug=False)

    xT_d = nc.dram_tensor("xT", [C, N], fm, kind="ExternalInput")
    wqkvT_d = nc.dram_tensor("wqkvT", [C, 3 * C], fm, kind="ExternalInput")
    wprojT_d = nc.dram_tensor("wprojT", [C, C], fm, kind="ExternalInput")
    bias_d = nc.dram_tensor("bias_rep", [128, C], f32, kind="ExternalInput")
    y_d = nc.dram_tensor("y", [N, C], f32, kind="ExternalOutput")

    with tile.TileContext(nc) as tc:
        with tc.tile_pool(name="pers", bufs=1) as pers:
            # Q^T,K^T tiles [d, n]: tile m holds heads 2m (parts 0:64) and
            # 2m+1 (parts 64:128); m 0..5 = Q, 6..11 = K.
            qkt = [pers.tile([128, N], fm, name=f"qkt{m}", tag=f"qkt{m}")
                   for m in range(MT_QK)]
            # V tiles [n-tile, pair, 130]: per pair block [V_h0 |1| V_h1 |1];
            # ones cols at 64 and 129 feed the denominator row of P@V.
            vbuf = [pers.tile([128, PAIRS, 130], fm, name=f"vbuf{i}", tag=f"vbuf{i}")
                    for i in range(NT)]

            # ---------------- phase A: QKV projection ----------------
            with tc.tile_pool(name="phA", bufs=1) as pA, \
                 tc.tile_pool(name="phA_ps", bufs=2, space="PSUM") as pAp:
                xt = [pA.tile([128, N], fm, name=f"xt{k}", tag=f"xt{k}")
                      for k in range(KT)]
                wq = [pA.tile([128, 3 * C], fm, name=f"wq{k}", tag=f"wq{k}")
                      for k in range(KT)]
                for k in range(KT):
                    nc.sync.dma_start(xt[k][:], xT_d[128 * k:128 * (k + 1), :])
                    nc.sync.dma_start(wq[k][:], wqkvT_d[128 * k:128 * (k + 1), :])
                for i in range(NT):
                    ones_ap = vbuf[i].rearrange("p a (t c) -> p a t c", c=65)[:, :, :, 64]
                    nc.vector.memset(ones_ap.bitcast(f32), 1.0)

                # Q^T / K^T: out[o-tile, n] = wqkvT[:, o-cols].T @ x^T
                for m in range(MT_QK):
                    ps = pAp.tile([128, N], f32, name="qk_ps", tag="qk_ps")
                    for j in range(2):
                        for k in range(KT):
                            mm(
                                ps[:, 512 * j:512 * (j + 1)],
                                wq[k][:, 128 * m:128 * (m + 1)],
                                xt[k][:, 512 * j:512 * (j + 1)],
                                start=(k == 0), stop=(k == KT - 1),
                                                            )
                    nc.scalar.copy(qkt[m][:], ps[:])

                # V natural: out[n-tile, o_v] = x^T[:, n-cols].T @ wqkvT[:, v-cols]
                for i in range(NT):
                    ps = pAp.tile([128, N], f32, name="v_ps", tag="v_ps")
                    for c0, w in ((0, 512), (512, 256)):
                        for k in range(KT):
                            mm(
                                ps[:, c0:c0 + w],
                                xt[k][:, 128 * i:128 * (i + 1)],
                                wq[k][:, 2 * C + c0:2 * C + c0 + w],
                                start=(k == 0), stop=(k == KT - 1),
                                                            )
                    v_view = ps[:, 0:C].rearrange("p (a t c) -> p a t c", t=2, c=64)
                    nc.vector.tensor_copy(vbuf[i][:, :, 0:64], v_view[:, :, 0, :])
                    nc.vector.tensor_copy(vbuf[i][:, :, 65:129], v_view[:, :, 1, :])

            # ---------------- phase B: attention + projection ----------------
            with tc.tile_pool(name="phB1", bufs=1) as pB1, \
                 tc.tile_pool(name="phB", bufs=2) as pB, \
                 tc.tile_pool(name="dramb", bufs=2, space="DRAM") as pDr, \
                 tc.tile_pool(name="s_ps_pool", bufs=2, space="PSUM") as pBs, \
                 tc.tile_pool(name="pv_ps_pool", bufs=2, space="PSUM") as pBpv:
                # attn_out^T tiles [c, n]; wproj/bias loaded into reused space.
                aot = [pB1.tile([128, N], fm, name=f"aot{t}", tag=f"aot{t}")
                       for t in range(PAIRS)]
                wp = [pB1.tile([128, C], fm, name=f"wp{k}", tag=f"wp{k}")
                      for k in range(KT)]
                bias_t = pB1.tile([128, C], f32, name="bias_t", tag="bias_t")
                for k in range(KT):
                    nc.sync.dma_start(wp[k][:], wprojT_d[128 * k:128 * (k + 1), :])
                nc.sync.dma_start(bias_t[:], bias_d[:])

                for t in range(PAIRS):
                    qt, kt = qkt[t], qkt[PAIRS + t]
                    pv_ps = [pBpv.tile([65, N], f32, name=f"pv{h}", tag="pv")
                             for h in range(2)]
                    for i in range(NT):
                        stexp = pB.tile([128, 2, N], fm, name="stexp", tag="stexp", bufs=3)
                        for j in range(2):
                            s_ps = pBs.tile([128, N], f32, name="s_ps", tag="s_ps")
                            for h in range(2):
                                # S^T[m, n] = sum_d K^T[d, m] * Q^T[d, n]
                                mm(
                                    s_ps[:, 512 * h:512 * (h + 1)],
                                    kt[64 * h:64 * (h + 1), 128 * i:128 * (i + 1)],
                                    qt[64 * h:64 * (h + 1), 512 * j:512 * (j + 1)],
                                    start=True, stop=True,
                                )
                            # exp(S^T / 8) for both heads in one pass, PSUM -> SBUF
                            nc.scalar.activation(
                                stexp[:, :, 512 * j:512 * (j + 1)],
                                s_ps[:, 0:N].rearrange("p (h n) -> p h n", h=2),
                                Exp, scale=SCALE,
                            )
                        for h in range(2):
                            for j in range(2):
                                # (exp(S^T) stacked with ones-col V): rows 0:64 are
                                # (P~ @ V)^T, row 64 is the softmax denominator.
                                mm(
                                    pv_ps[h][:, 512 * j:512 * (j + 1)],
                                    vbuf[i][:, t, 65 * h:65 * (h + 1)],
                                    stexp[:, h, 512 * j:512 * (j + 1)],
                                    start=(i == 0), stop=(i == NT - 1),
                                                                    )
                    for h in range(2):
                        stage = pB.tile([65, N], f32, name="stage", tag="stage")
                        nc.vector.tensor_copy(stage[:], pv_ps[h][:])
                        # Reciprocal of the denominator row. A [1, N] DVE
                        # reciprocal costs ~6.5us (iterative divide is
                        # FD-bound), so bounce the row through DRAM into a
                        # [128, N//128] layout where the same op is ~130ns.
                        dr = pDr.tile([1, N], f32, name="dr", tag="dr")
                        nc.sync.dma_start(dr[:], stage[64:65, :])
                        den_t = pB.tile([128, N // 128], f32, name="den_t", tag="den_t")
                        nc.sync.dma_start(
                            den_t[:], dr[:].rearrange("p (a b) -> (p a) b", a=128))
                        nc.vector.reciprocal(den_t[:], den_t[:])
                        dr2 = pDr.tile([1, N], f32, name="dr2", tag="dr2")
                        nc.sync.dma_start(
                            dr2[:].rearrange("p (a b) -> (p a) b", a=128), den_t[:])
                        # partition-broadcast of the reciprocal row: SBUF APs
                        # can't have zero partition step, so broadcast from DRAM.
                        rb = pB.tile([64, N], f32, name="rb", tag="rb")
                        nc.sync.dma_start(rb[:], dr2[:].to_broadcast((64, N)))
                        if h == 0:
                            nc.vector.tensor_mul(aot[t][0:64, :], stage[0:64, :], rb[:])
                        else:
                            tmp = pB.tile([64, N], fm, name="tmp1", tag="tmp1")
                            nc.vector.tensor_mul(tmp[:], stage[0:64, :], rb[:])
                            # DVE lanes cannot shift partitions; DMA moves the
                            # odd head's rows into partitions 64:128.
                            nc.sync.dma_start(aot[t][64:128, :], tmp[:])

                # output projection: y[n, o] = attn_out^T.T @ w_proj^T + b
                for i in range(NT):
                    yt = pB.tile([128, C], f32, name="yt", tag="yt")
                    for c0 in (0, 384):
                        pp = pBs.tile([128, 384], f32, name="pp", tag="s_ps")
                        for k in range(KT):
                            mm(
                                pp[:, 0:384],
                                aot[k][:, 128 * i:128 * (i + 1)],
                                wp[k][:, c0:c0 + 384],
                                start=(k == 0), stop=(k == KT - 1),
                                                            )
                        nc.vector.tensor_add(yt[:, c0:c0 + 384], pp[:, 0:384],
                                             bias_t[:, c0:c0 + 384])
                    nc.sync.dma_start(y_d[128 * i:128 * (i + 1), :], yt[:])

    nc.compile()
    return nc


def round_f32r(a):
    """Round fp32 to the FP32r grid (11 explicit mantissa bits, RNE) --
    what the PE reads for float32r matmuls."""
    a = np.ascontiguousarray(a, dtype=np.float32)
    b = a.view(np.uint32)
    r = (b + np.uint32(0x7FF) + ((b >> np.uint32(12)) & np.uint32(1))) \
        & np.uint32(0xFFFFF000)
    return r.view(np.float32)


def make_in_maps(x, w_qkv, w_proj, b_proj):
    wqkvT = round_f32r(np.asarray(w_qkv, dtype=np.float32).T)
    wprojT = round_f32r(np.asarray(w_proj, dtype=np.float32).T)
    bias_rep = np.ascontiguousarray(
        np.broadcast_to(np.asarray(b_proj, dtype=np.float32), (128, C)))
    x = np.asarray(x, dtype=np.float32)
    return [
        {
            "xT": round_f32r(x[b].T),
            "wqkvT": wqkvT,
            "wprojT": wprojT,
            "bias_rep": bias_rep,
        }
        for b in range(B)
    ]


def kernel(x, w_qkv, w_proj, b_proj):
    from concourse.bass_utils import run_bass_kernel_spmd

    if "nc" not in _CACHE:
        _CACHE["nc"] = build_program()
    nc = _CACHE["nc"]

    in_maps = make_in_maps(x, w_qkv, w_proj, b_proj)
    res = run_bass_kernel_spmd(nc, in_maps, core_ids=list(range(B)))
    out = np.stack([res.results[b]["y"] for b in range(B)], axis=0)
    return out.astype(np.float32)


# revision 13
# speedup vs baseline: 3.0143x; 1.0260x over previous
"""Multi-head attention (B=8, N=1024, C=768, H=12) on 8 Trainium2 NeuronCores.

Sharding: data-parallel, one batch element per core. Each core computes the
full attention block for its batch: QKV projection, per-head softmax(QK^T/8)V,
and the output projection, entirely on-chip (SBUF/PSUM).

Layout strategy (chosen so no on-device transposes are needed):
  - host passes x^T [C, N], w_qkv^T [C, 3C], w_proj^T [C, C], bias replicated
    to [128, C].
  - Q, K are produced transposed ([d, n], head-dim on partitions) by the QKV
    matmul; V is produced in natural [n, d] layout by swapping lhsT/rhs.
  - scores are computed transposed (S^T[m, n] = K Q^T) so that exp(S^T) can be
    consumed directly as the moving operand of the P@V matmul.
  - V tiles carry an appended ones-column, so the P@V matmul's 65th output row
    is the softmax denominator (row-sum of exp scores) for free.
  - normalization multiplies by a reciprocal row broadcast across partitions
    via a DRAM-bounced DMA (SBUF APs cannot partition-broadcast).

Matmul operands use dtype float32r: single-pass PE streaming (1 column/cycle,
4x faster than float32's two-pass LOW/HIGH emulation) with 11 explicit
mantissa bits. Producers round on write; DRAM inputs are pre-rounded on host.
"""

import sys

import numpy as np

if "/opt/trn_rl_repo" not in sys.path:
    sys.path.insert(0, "/opt/trn_rl_repo")

B = 8
N = 1024
C = 768
H = 12
D = 64
SCALE = D ** -0.5
KT = C // 128           # 6 contraction tiles over channels
MT_QK = 2 * C // 128    # 12 output tiles for Q and K (o in [0, 1536))
NT = N // 128           # 8 token tiles
PAIRS = H // 2          # 6 head pairs

_CACHE = {}


def build_program(fast=True):
    import concourse.bacc as bacc
    import concourse.mybir as mybir
    import concourse.tile as tile

    f32 = mybir.dt.float32
    f32r = mybir.dt.float32r
    Exp = mybir.ActivationFunctionType.Exp
    fm = f32r if fast else f32

    nc = bacc.Bacc("TRN2", target_bir_lowering=False, debug=False)

    xT_d = nc.dram_tensor("xT", [C, N], fm, kind="ExternalInput")
    wqkvT_d = nc.dram_tensor("wqkvT", [C, 3 * C], fm, kind="ExternalInput")
    wprojT_d = nc.dram_tensor("wprojT", [C, C], fm, kind="ExternalInput")
    bias_d = nc.dram_tensor("bias_rep", [128, C], f32, kind="ExternalInput")
    y_d = nc.dram_tensor("y", [N, C], f32, kind="ExternalOutput")

    mm = nc.tensor.matmul

    with tile.TileContext(nc) as tc:
        with tc.tile_pool(name="pers", bufs=1) as pers:
            # Q^T,K^T tiles [d, n]: tile m holds heads 2m (parts 0:64) and
            # 2m+1 (parts 64:128); m 0..5 = Q, 6..11 = K.
            qkt = [pers.tile([128, N], fm, name=f"qkt{m}", tag=f"qkt{m}")
                   for m in range(MT_QK)]
            # V tiles [n-tile, pair, 130]: per pair block [V_h0 |1| V_h1 |1];
            # ones cols at 64 and 129 feed the denominator row of P@V.
            vbuf = [pers.tile([128, PAIRS, 130], fm, name=f"vbuf{i}", tag=f"vbuf{i}")
                    for i in range(NT)]

            # ---------------- phase A: QKV projection ----------------
            with tc.tile_pool(name="phA", bufs=1) as pA, \
                 tc.tile_pool(name="phA_ps", bufs=2, space="PSUM") as pAp:
                xt = [pA.tile([128, N], fm, name=f"xt{k}", tag=f"xt{k}")
                      for k in range(KT)]
                wqk = [pA.tile([128, 2 * C], fm, name=f"wqk{k}", tag=f"wqk{k}")
                       for k in range(KT)]
                wv = [pA.tile([128, C], fm, name=f"wv{k}", tag=f"wv{k}")
                      for k in range(KT)]
                for k in range(KT):
                    nc.sync.dma_start(xt[k][:], xT_d[128 * k:128 * (k + 1), :])
                for k in range(KT):
                    nc.sync.dma_start(wqk[k][:],
                                      wqkvT_d[128 * k:128 * (k + 1), 0:2 * C])
                for k in range(KT):
                    nc.sync.dma_start(wv[k][:],
                                      wqkvT_d[128 * k:128 * (k + 1), 2 * C:3 * C])
                for i in range(NT):
                    ones_ap = vbuf[i].rearrange("p a (t c) -> p a t c", c=65)[:, :, :, 64]
                    nc.vector.memset(ones_ap.bitcast(f32), 1.0)

                def emit_qk(m):
                    ps = pAp.tile([128, N], f32, name="qk_ps", tag="qk_ps")
                    for j in range(2):
                        for k in range(KT):
                            mm(ps[:, 512 * j:512 * (j + 1)],
                               wqk[k][:, 128 * m:128 * (m + 1)],
                               xt[k][:, 512 * j:512 * (j + 1)],
                               start=(k == 0), stop=(k == KT - 1))
                    nc.vector.tensor_copy(qkt[m][:], ps[:])

                def emit_v(i):
                    ps = pAp.tile([128, N], f32, name="v_ps", tag="v_ps")
                    for c0, w in ((0, 512), (512, 256)):
                        for k in range(KT):
                            mm(ps[:, c0:c0 + w],
                               xt[k][:, 128 * i:128 * (i + 1)],
                               wv[k][:, c0:c0 + w],
                               start=(k == 0), stop=(k == KT - 1))
                    v_view = ps[:, 0:C].rearrange("p (a t c) -> p a t c", t=2, c=64)
                    nc.vector.tensor_copy(vbuf[i][:, :, 0:64], v_view[:, :, 0, :])
                    nc.vector.tensor_copy(vbuf[i][:, :, 65:129], v_view[:, :, 1, :])

                # head pairs 0/1 first so attention can overlap the QKV tail
                for m in (0, 6, 1, 7):
                    emit_qk(m)
                for i in range(NT):
                    emit_v(i)
                for m in (2, 8, 3, 9, 4, 10, 5, 11):
                    emit_qk(m)

            # ---------------- phase B: attention + projection ----------------
            with tc.tile_pool(name="phB1", bufs=1) as pB1, \
                 tc.tile_pool(name="phB", bufs=2) as pB, \
                 tc.tile_pool(name="dramb", bufs=2, space="DRAM") as pDr, \
                 tc.tile_pool(name="s_ps_pool", bufs=2, space="PSUM") as pBs, \
                 tc.tile_pool(name="pv_ps_pool", bufs=2, space="PSUM") as pBpv:
                # attn_out^T tiles [c, n]; wproj/bias loaded into reused space.
                aot = [pB1.tile([128, N], fm, name=f"aot{t}", tag=f"aot{t}")
                       for t in range(PAIRS)]
                wp = [pB1.tile([128, C], fm, name=f"wp{k}", tag=f"wp{k}")
                      for k in range(KT)]
                bias_t = pB1.tile([128, C], f32, name="bias_t", tag="bias_t")
                for k in range(KT):
                    nc.sync.dma_start(wp[k][:], wprojT_d[128 * k:128 * (k + 1), :])
                nc.sync.dma_start(bias_t[:], bias_d[:])

                for t in range(PAIRS):
                    qt, kt = qkt[t], qkt[PAIRS + t]
                    pv_ps = [pBpv.tile([65, N], f32, name=f"pv{h}", tag="pv")
                             for h in range(2)]
                    for i in range(NT):
                        stexp = pB.tile([128, 2, N], fm, name="stexp", tag="stexp",
                                        bufs=3)
                        for j in range(2):
                            s_ps = pBs.tile([128, N], f32, name="s_ps", tag="s_ps")
                            for h in range(2):
                                # S^T[m, n] = sum_d K^T[d, m] * Q^T[d, n];
                                # h0/h1 hit distinct PE row groups and run
                                # concurrently.
                                mm(s_ps[:, 512 * h:512 * (h + 1)],
                                   kt[64 * h:64 * (h + 1), 128 * i:128 * (i + 1)],
                                   qt[64 * h:64 * (h + 1), 512 * j:512 * (j + 1)],
                                   start=True, stop=True)
                            # exp(S^T / 8) for both heads in one pass, PSUM->SBUF
                            nc.scalar.activation(
                                stexp[:, :, 512 * j:512 * (j + 1)],
                                s_ps[:, 0:N].rearrange("p (h n) -> p h n", h=2),
                                Exp, scale=SCALE)
                        for h in range(2):
                            for j in range(2):
                                # rows 0:64 = (P~ @ V)^T, row 64 = denominator
                                mm(pv_ps[h][:, 512 * j:512 * (j + 1)],
                                   vbuf[i][:, t, 65 * h:65 * (h + 1)],
                                   stexp[:, h, 512 * j:512 * (j + 1)],
                                   start=(i == 0), stop=(i == NT - 1))
                    for h in range(2):
                        stage = pB.tile([65, N], f32, name="stage", tag="stage")
                        nc.vector.tensor_copy(stage[:], pv_ps[h][:])
                        # Reciprocal of the denominator row: a [1, N] DVE
                        # reciprocal is FD-bound (~6.5us), so DMA the row into
                        # [128, N/128] first where the same op is ~200ns.
                        den_t = pB.tile([128, N // 128], f32, name="den_t",
                                        tag="den_t")
                        nc.sync.dma_start(den_t[:], stage[64:65, :])
                        nc.vector.reciprocal(den_t[:], den_t[:])
                        dr2 = pDr.tile([1, N], f32, name="dr2", tag="dr2")
                        nc.sync.dma_start(
                            dr2[:].rearrange("p (a b) -> (p a) b", a=128), den_t[:])
                        # partition-broadcast of the reciprocal row: SBUF APs
                        # can't have zero partition step, so broadcast from DRAM.
                        rb = pB.tile([64, N], f32, name="rb", tag="rb")
                        nc.sync.dma_start(rb[:], dr2[:].to_broadcast((64, N)))
                        if h == 0:
                            nc.vector.tensor_mul(aot[t][0:64, :], stage[0:64, :], rb[:])
                        else:
                            tmp = pB.tile([64, N], fm, name="tmp1", tag="tmp1")
                            nc.vector.tensor_mul(tmp[:], stage[0:64, :], rb[:])
                            # DVE lanes cannot shift partitions; DMA moves the
                            # odd head's rows into partitions 64:128.
                            nc.sync.dma_start(aot[t][64:128, :], tmp[:])

                # output projection: y[n, o] = attn_out^T.T @ w_proj^T + b
                for i in range(NT):
                    yt = pB.tile([128, C], f32, name="yt", tag="yt")
                    for c0 in (0, 384):
                        # alternate the two released attention psum pools so
                        # four k-accumulation groups can be in flight
                        if (2 * i + c0 // 384) % 2 == 0:
                            pp = pBs.tile([128, 384], f32, name="pp", tag="s_ps")
                        else:
                            pp = pBpv.tile([128, 384], f32, name="pp", tag="pv")
                        for k in range(KT):
                            mm(pp[:, 0:384],
                               aot[k][:, 128 * i:128 * (i + 1)],
                               wp[k][:, c0:c0 + 384],
                               start=(k == 0), stop=(k == KT - 1))
                        nc.vector.tensor_add(yt[:, c0:c0 + 384], pp[:, 0:384],
                                             bias_t[:, c0:c0 + 384])
                    nc.sync.dma_start(y_d[128 * i:128 * (i + 1), :], yt[:])

    nc.compile()
    return nc


def round_f32r(a):
    """Round fp32 to the FP32r grid (11 explicit mantissa bits, RNE) --
    what the PE reads for float32r matmuls."""
    a = np.ascontiguousarray(a, dtype=np.float32)
    b = a.view(np.uint32)
    r = (b + np.uint32(0x7FF) + ((b >> np.uint32(12)) & np.uint32(1))) \
        & np.uint32(0xFFFFF000)
    return r.view(np.float32)


def make_in_maps(x, w_qkv, w_proj, b_proj):
    wqkvT = round_f32r(np.asarray(w_qkv, dtype=np.float32).T)
    wprojT = round_f32r(np.asarray(w_proj, dtype=np.float32).T)
    bias_rep = np.ascontiguousarray(
        np.broadcast_to(np.asarray(b_proj, dtype=np.float32), (128, C)))
    x = np.asarray(x, dtype=np.float32)
    return [
        {
            "xT": round_f32r(x[b].T),
            "wqkvT": wqkvT,
            "wprojT": wprojT,
            "bias_rep": bias_rep,
        }
        for b in range(B)
    ]


def kernel(x, w_qkv, w_proj, b_proj):
    from concourse.bass_utils import run_bass_kernel_spmd

    if "nc" not in _CACHE:
        _CACHE["nc"] = build_program()
    nc = _CACHE["nc"]

    in_maps = make_in_maps(x, w_qkv, w_proj, b_proj)
    res = run_bass_kernel_spmd(nc, in_maps, core_ids=list(range(B)))
    out = np.stack([res.results[b]["y"] for b in range(B)], axis=0)
    return out.astype(np.float32)
